# revision 3
# baseline (speedup 1.0000x reference)
"""DPLookupRenderer Trainium2 kernel.

kernel(dp_comb, iuv_image_b) -> [4,16,1024,1024] f32

Sharding: 8 cores = 4 batches x 2 image halves (H split). Each core gets its
batch's atlas and half the iuv image, and produces [16,512,1024].

Per-core device program:
  Phase 1: repack the atlas into a DRAM lookup table tbl[(k,x,y), c] fp16
           (k = part id 0..23, 128x128 patch, 16 channels innermost).
  Phase 2: stream pixels in chunks of 128*F; per pixel compute part id,
           bilinear coords and a table index; one indirect DMA per chunk
           gathers 2 runs of 32 fp16 values per pixel (table rows x0 and
           x0+1, each covering y0,y0+1 for all 16 channels); DVE applies
           the bilinear weights; output is written in [c, px] order.
"""

import sys

sys.path.insert(0, '/opt/trn_rl_repo')

import numpy as np

import bass_rust
import concourse.bass as bass
import concourse.mybir as mybir
import concourse.tile as tile
from concourse.bass import IndirectOffsetOnAxis

F32 = mybir.dt.float32
F16 = mybir.dt.float16
I32 = mybir.dt.int32
ALU = mybir.AluOpType
AF = mybir.ActivationFunctionType

C = 16
P = 128
HH = 512
W = 1024
NPX = HH * W
NENT = 24 * P * P
N_CORES = 8

# ---------------------------------------------------------------------------
# walrus in this env rejects instructions with more than ~2 sync waits
# ("Too many sync wait commands"); split waits onto nop carriers.

MAX_WAITS = 1


def _drain_and_barrier_split(self, tick_clock, wait_clock):
    nc = self.nc
    carrier = nc.sync.nop(nofuse=True)
    wait_clock.add_sem_waits(
        carrier.ins, bass_rust.ScopedClock({None: tick_clock.global_clock})
    )
    si = carrier.ins.sync_info
    if si is not None and len(si.on_wait) > MAX_WAITS:
        extra = list(si.on_wait[MAX_WAITS:])
        del si.on_wait[MAX_WAITS:]
        for i in range(0, len(extra), MAX_WAITS):
            n = nc.sync.nop(nofuse=True)
            n.ins.sync_info = mybir.SyncInfo(
                on_wait=list(extra[i : i + MAX_WAITS]), on_update=[]
            )
    nc.sync.drain()
    nc.all_engine_barrier()
    assert self.sems is not None
    popped = nc._tile_sem_poison_stack.pop()
    assert popped is self._sem_poison
    nc.clear_and_free_semaphores(list(self.sems.allocated().values()))
    nc.all_engine_barrier()


tile.TileContext._drain_and_barrier = _drain_and_barrier_split

_uid = [0]


def split_waits(nc, maxw=MAX_WAITS):
    for f in nc.m.functions:
        for bb in f.blocks:
            out = []
            for ins in bb.instructions:
                si = ins.sync_info
                if si is not None and len(si.on_wait) > maxw:
                    assert ins.engine is not None, ins.name
                    extra = list(si.on_wait[: len(si.on_wait) - maxw])
                    del si.on_wait[: len(si.on_wait) - maxw]
                    for i in range(0, len(extra), maxw):
                        _uid[0] += 1
                        n = mybir.InstNoOp(
                            name=f"waitcarrier-{_uid[0]}",
                            ins=[],
                            outs=[],
                            engine=ins.engine,
                        )
                        n.sync_info = mybir.SyncInfo(
                            on_wait=list(extra[i : i + maxw]), on_update=[]
                        )
                        nc.register_instruction(n, overwrite=True)
                        out.append(n)
                out.append(ins)
            bb.instructions = out


# ---------------------------------------------------------------------------


def build(F=256, tbl_dt=F16):
    CH = P * F
    nchunks = NPX // CH
    assert NPX % CH == 0

    nc = bass.Bass()
    dp = nc.dram_tensor("dp", [C, 4 * P, 6 * P], F32, kind="ExternalInput")
    iuv = nc.dram_tensor("iuv", [3, HH, W], F32, kind="ExternalInput")
    out = nc.dram_tensor("out", [C, HH, W], F32, kind="ExternalOutput")
    tbl = nc.dram_tensor("tbl", [NENT, C], tbl_dt)

    # Pool-engine DMA lowering happens at TileContext exit; disable the
    # conservative SWDGE desc-count check (the ring is drained with flow
    # control).
    nc.gpsimd.enable_hardware_checks = False
    with tile.TileContext(nc) as tc:
        # ---------------- Phase 1: table build ----------------
        with tc.tile_pool(name="bld", bufs=2) as bp:
            for k in range(24):
                rk, ck = divmod(k, 6)
                patch = bp.tile([P, C, P], tbl_dt, tag="patch")
                src = dp[:, rk * P:(rk + 1) * P, ck * P:(ck + 1) * P]
                nc.gpsimd.dma_start(out=patch[:], in_=src.rearrange("c x y -> x c y"))
                patchT = bp.tile([P, P, C], tbl_dt, tag="patchT")
                nc.vector.tensor_copy(
                    out=patchT[:], in_=patch[:].rearrange("x c y -> x y c")
                )
                dst = tbl[k * P * P:(k + 1) * P * P, :].rearrange(
                    "(x y) c -> x y c", x=P
                )
                nc.sync.dma_start(out=dst, in_=patchT[:])

        # ---------------- Phase 2: pixel loop ----------------
        iuv_v = iuv.rearrange("c h w -> c (h w)").rearrange(
            "c (n p f) -> c n p f", p=P, f=F
        )
        out_v = out.rearrange("c h w -> c (h w)")

        with tc.tile_pool(name="px", bufs=2) as pp, tc.tile_pool(name="g", bufs=2) as gp:
            for n in range(nchunks):
                pid = pp.tile([P, F], F32, tag="pid")
                u = pp.tile([P, F], F32, tag="u")
                v = pp.tile([P, F], F32, tag="v")
                nc.sync.dma_start(out=pid[:], in_=iuv_v[0, n])
                nc.sync.dma_start(out=u[:], in_=iuv_v[1, n])
                nc.sync.dma_start(out=v[:], in_=iuv_v[2, n])

                # valid = (pid >= 0.5) * (pid <= 24.5)
                va = pp.tile([P, F], F32, tag="va")
                vb = pp.tile([P, F], F32, tag="vb")
                valid = pp.tile([P, F], F32, tag="valid")
                nc.vector.tensor_single_scalar(va[:], pid[:], 0.5, ALU.is_ge)
                nc.vector.tensor_single_scalar(vb[:], pid[:], 24.5, ALU.is_le)
                nc.vector.tensor_tensor(valid[:], va[:], vb[:], ALU.mult)

                # k = clip(pid - 1, 0, 23)
                kk = pp.tile([P, F], F32, tag="kk")
                nc.vector.tensor_scalar(kk[:], pid[:], 1.0, 0.0, ALU.subtract, ALU.max)
                nc.vector.tensor_single_scalar(kk[:], kk[:], 23.0, ALU.min)

                # fx = 63.5*u + 63.5 ; fy = 63.5*v + 63.5   (on ACT engine)
                fx = pp.tile([P, F], F32, tag="fx")
                fy = pp.tile([P, F], F32, tag="fy")
                nc.scalar.activation(fx[:], u[:], AF.Copy, bias=63.5, scale=63.5)
                nc.scalar.activation(fy[:], v[:], AF.Copy, bias=63.5, scale=63.5)

                # floor + frac (robust to round-nearest or truncating casts)
                def floor_frac(fsrc, tagp):
                    xi = pp.tile([P, F], I32, tag=tagp + "i")
                    xf = pp.tile([P, F], F32, tag=tagp + "f")
                    gt = pp.tile([P, F], F32, tag=tagp + "g")
                    wfrac = pp.tile([P, F], F32, tag=tagp + "w")
                    nc.vector.tensor_copy(xi[:], fsrc[:])
                    nc.vector.tensor_copy(xf[:], xi[:])
                    nc.vector.tensor_tensor(gt[:], xf[:], fsrc[:], ALU.is_gt)
                    nc.vector.tensor_tensor(xf[:], xf[:], gt[:], ALU.subtract)
                    nc.vector.tensor_tensor(wfrac[:], fsrc[:], xf[:], ALU.subtract)
                    return xf, wfrac

                x0f, wx = floor_frac(fx, "x")
                y0f, wy = floor_frac(fy, "y")

                # idx = k*16384 + x0*128 + y0 ; second row at idx + 128
                t1 = pp.tile([P, F], F32, tag="t1")
                idxf = pp.tile([P, F], F32, tag="idxf")
                nc.vector.scalar_tensor_tensor(
                    t1[:], x0f[:], 128.0, y0f[:], ALU.mult, ALU.add
                )
                nc.vector.scalar_tensor_tensor(
                    idxf[:], kk[:], 16384.0, t1[:], ALU.mult, ALU.add
                )
                idx = pp.tile([P, F, 2], I32, tag="idx")
                nc.vector.tensor_copy(idx[:, :, 0:1].squeeze(2), idxf[:])
                nc.vector.tensor_single_scalar(idxf[:], idxf[:], 128.0, ALU.add)
                nc.vector.tensor_copy(idx[:, :, 1:2].squeeze(2), idxf[:])

                # weights (valid folded into the x pair)
                wxl = pp.tile([P, F], F32, tag="wxl")
                wyl = pp.tile([P, F], F32, tag="wyl")
                nc.vector.tensor_scalar(wxl[:], wx[:], -1.0, 1.0, ALU.mult, ALU.add)
                nc.vector.tensor_scalar(wyl[:], wy[:], -1.0, 1.0, ALU.mult, ALU.add)
                nc.vector.tensor_tensor(wxl[:], wxl[:], valid[:], ALU.mult)
                nc.vector.tensor_tensor(wx[:], wx[:], valid[:], ALU.mult)
                w00 = pp.tile([P, F], F32, tag="w00")
                w01 = pp.tile([P, F], F32, tag="w01")
                w10 = pp.tile([P, F], F32, tag="w10")
                w11 = pp.tile([P, F], F32, tag="w11")
                nc.vector.tensor_tensor(w00[:], wxl[:], wyl[:], ALU.mult)
                nc.vector.tensor_tensor(w01[:], wxl[:], wy[:], ALU.mult)
                nc.vector.tensor_tensor(w10[:], wx[:], wyl[:], ALU.mult)
                nc.vector.tensor_tensor(w11[:], wx[:], wy[:], ALU.mult)

                # gather: per index 32 contiguous fp16 (2 entries x 16 ch).
                # The DGE on this toolchain only honours one offset per
                # partition per instruction ([P,1] offset column), so issue
                # one indirect DMA per (pixel-slot, row) pair.
                G = gp.tile([P, F, 64], tbl_dt, tag="G")
                for f in range(F):
                    for j in range(2):
                        nc.gpsimd.indirect_dma_start(
                            out=G[:, f, j * 32:(j + 1) * 32],
                            out_offset=None,
                            in_=tbl[:],
                            in_offset=IndirectOffsetOnAxis(
                                ap=idx[:, f, j:j + 1], axis=0
                            ),
                        )

                # lerp: O[p,c,f] = sum_t w_t[p,f] * G[p,f,t*16+c]
                O = gp.tile([P, C, F], F32, tag="O")
                T = gp.tile([P, C, F], F32, tag="T")
                Ov = O[:].rearrange("p c f -> p f c")
                Tv = T[:].rearrange("p c f -> p f c")

                def wb(t):
                    return t[:].unsqueeze(2).to_broadcast([P, F, C])

                nc.vector.tensor_tensor(Ov, G[:, :, 0:16], wb(w00), ALU.mult)
                nc.vector.tensor_tensor(Tv, G[:, :, 16:32], wb(w01), ALU.mult)
                nc.vector.tensor_add(O[:], O[:], T[:])
                nc.vector.tensor_tensor(Tv, G[:, :, 32:48], wb(w10), ALU.mult)
                nc.vector.tensor_add(O[:], O[:], T[:])
                nc.vector.tensor_tensor(Tv, G[:, :, 48:64], wb(w11), ALU.mult)
                nc.vector.tensor_add(O[:], O[:], T[:])

                dst = out_v[:, n * CH:(n + 1) * CH].rearrange("c (p f) -> p c f", p=P)
                nc.sync.dma_start(out=dst, in_=O[:])

    split_waits(nc)
    return nc


# ---------------------------------------------------------------------------
# Runner: mirror of concourse.bass2jax.run_bass_via_pjrt's multi-core path,
# kept as a reusable callable so repeated (timed) runs don't recompile.

_CACHED = {}


def make_runner():
    import jax
    from jax.sharding import Mesh, PartitionSpec
    from jax.experimental.shard_map import shard_map
    from concourse import bass2jax

    nc = build()
    bass2jax.install_neuronx_cc_hook()

    partition_name = nc.partition_id_tensor.name if nc.partition_id_tensor else None
    in_names, out_names, out_avals, zero_outs = [], [], [], []
    for alloc in nc.m.functions[0].allocations:
        if not isinstance(alloc, mybir.MemoryLocationSet):
            continue
        name = alloc.memorylocations[0].name
        if alloc.kind == "ExternalInput":
            if name != partition_name:
                in_names.append(name)
        elif alloc.kind == "ExternalOutput":
            shape = tuple(alloc.tensor_shape)
            dtype = mybir.dt.np(alloc.dtype)
            out_names.append(name)
            out_avals.append(jax.core.ShapedArray(shape, dtype))
            zero_outs.append(np.zeros(shape, dtype))
    n_params = len(in_names)
    n_outs = len(out_avals)
    all_in_names = list(in_names) + list(out_names)
    if partition_name is not None:
        all_in_names.append(partition_name)

    def _body(*args):
        operands = list(args)
        if partition_name is not None:
            operands.append(bass2jax.partition_id_tensor())
        outs = bass2jax._bass_exec_p.bind(
            *operands,
            out_avals=tuple(out_avals),
            in_names=tuple(all_in_names),
            out_names=tuple(out_names),
            lowering_input_output_aliases=(),
            sim_require_finite=True,
            sim_require_nnan=True,
            nc=nc,
        )
        return tuple(outs)

    devices = jax.devices()[:N_CORES]
    mesh = Mesh(np.asarray(devices), ("core",))
    in_specs = (PartitionSpec("core"),) * (n_params + n_outs)
    out_specs = (PartitionSpec("core"),) * n_outs
    sharded = jax.jit(
        shard_map(
            _body, mesh=mesh, in_specs=in_specs, out_specs=out_specs, check_rep=False
        ),
        donate_argnums=tuple(range(n_params, n_params + n_outs)),
        keep_unused=True,
    )
    return {
        "fn": sharded,
        "in_names": in_names,
        "out_names": out_names,
        "zero_outs": zero_outs,
        "mesh": mesh,
        "n_params": n_params,
    }


def get_runner():
    if "r" not in _CACHED:
        _CACHED["r"] = make_runner()
    return _CACHED["r"]


def shard_inputs(dp_comb, iuv_image_b):
    """Concatenated global inputs for the 8-core shard_map call."""
    dp_comb = np.asarray(dp_comb, dtype=np.float32)
    iuv = np.asarray(iuv_image_b, dtype=np.float32)
    dps, iuvs = [], []
    for i in range(N_CORES):
        b, h = divmod(i, 2)
        dps.append(dp_comb[b])
        iuvs.append(iuv[b, :, h * HH:(h + 1) * HH, :])
    return {
        "dp": np.concatenate(dps, axis=0),
        "iuv": np.concatenate(iuvs, axis=0),
    }


def run_device(global_ins):
    r = get_runner()
    ins = [global_ins[name] for name in r["in_names"]]
    zeros = [
        np.zeros((N_CORES * z.shape[0], *z.shape[1:]), z.dtype)
        for z in r["zero_outs"]
    ]
    out_arrs = r["fn"](*ins, *zeros)
    return {
        name: np.asarray(out_arrs[i]) for i, name in enumerate(r["out_names"])
    }


def kernel(dp_comb, iuv_image_b):
    global_ins = shard_inputs(dp_comb, iuv_image_b)
    outs = run_device(global_ins)
    o = outs["out"].reshape(N_CORES, C, HH, W)
    full = np.empty((4, C, 2 * HH, W), dtype=np.float32)
    for i in range(N_CORES):
        b, h = divmod(i, 2)
        full[b, :, h * HH:(h + 1) * HH, :] = o[i]
    return full
